# revision 41
# baseline (speedup 1.0000x reference)
"""Trainium2 Bass kernel for nn_Entropy (histogram_binning): per-pixel Shannon
entropy of a 5x5-window KDE histogram over 256 intensity bins.

Math (validated in numpy to 2.1e-3 max rel err vs f64 oracle):
  k(x,b) = sigmoid'(10(x-b)) = 0.25*(1 - tanh^2(5(x-b)))
  Scale factors cancel in p = q/S, so we use m = 1 - t^2 directly.
  q[h,w,b] = 5x5 window sum of m = cnt(h,w) - winsum5x5(t^2)
  E = -sum_b p ln p = ln(S) - (sum_b q ln(q+EPS))/S,  S = sum_b q
  S comes analytically per pixel from 5 taps of 1-tanh^2 around frac(x)
  (range-masked), then a 5x5 window sum (H via matmul, W via shifted adds).

Pipeline per image (layout: partitions = h, free = (w-block, 256 bins)):
  MM1 (TensorE, fp16): d = 5u + 5n - 5b via stationary [4x 5u^T; 4x 5n^T;
    ones] and a delta-selector moving constant -> PSUM f32. The u/n split
    keeps d exact in fp16 where |d| is small (n,b integers are fp16-exact).
  tanh (ScalarE): PSUM -> SBUF fp16.
  square (DVE, fp16 2x): t^2 -> stripe [97, 100 blocks x 256] with 2+2
    zero-pad w-blocks; row 96 = in-range-w indicator (cw row).
  MM2 (TensorE, fp16, 5 shifted taps accumulated in PSUM): stationary
    [-band(96x96); ch(h)] x stripe -> q = cnt - winsum5x5(t^2) directly in
    PSUM. No scan, no transpose, no separate W pass.
  backend: Ln(q+EPS) (ScalarE, PSUM src), then tensor_tensor_reduce
    (DVE) computes sum_b q*ln per w column -> QL.
  E = lnS - QL/S on [96, 288] tiles.

ScalarE activations are batched per image phase (tanh batch / Ln batch) to
avoid activation-table thrash. Sharding: B*C = 24 images, 3 per core on 8
cores; no collectives. Self-contained; compiled once per process.
"""

import sys

sys.path.insert(0, "/opt/trn_rl_repo")

import numpy as np

H = 96
W = 96
NB = 256
NIMG = 3
NCORES = 8
EPS = 1e-10
WG = 4                 # w's per MM1 group
NG = W // WG           # 24 MM1 groups per image
NBLK = W + 4           # stripe w-blocks incl 2+2 pads
SCOLS = NBLK * NB      # 25600 stripe cols
NW = NIMG * W          # 288
BACKEND = "stt"        # "stt" | "ttr" | "reduce"

_CACHE = {}


def _build_consts():
    # MM1 moving selector [9, WG*NB] fp16: col (w', b) picks stationary rows
    # w' (5u) and 4+w' (5n), plus -5b via the ones row.
    sel = np.zeros((9, WG * NB), dtype=np.float32)
    b = np.arange(NB, dtype=np.float32)
    for j in range(WG):
        sel[j, j * NB:(j + 1) * NB] = 1.0
        sel[4 + j, j * NB:(j + 1) * NB] = 1.0
    sel[8, :] = np.tile(-5.0 * b, WG)

    hh = np.arange(H)
    band = (np.abs(hh[:, None] - hh[None, :]) <= 2).astype(np.float32)
    bandch = np.zeros((H + 1, H), dtype=np.float32)
    bandch[:H, :] = -band
    bandch[H, :] = band.sum(0)  # ch(h) in {3,4,5}
    return sel.astype(np.float16), bandch.astype(np.float16)


def _emit_kernel(nc, tc, ctx, ins, outs):
    from concourse import mybir

    f32 = mybir.dt.float32
    f16 = mybir.dt.float16
    AF = mybir.ActivationFunctionType
    OP = mybir.AluOpType

    u_d, nf_d, st_d, sel_d, bandch_d, padz_d, ones_d, twos_d = ins
    (ent_d,) = outs

    consts = ctx.enter_context(tc.tile_pool(name="consts", bufs=1))
    sm = ctx.enter_context(tc.tile_pool(name="sm", bufs=1))
    tpool = ctx.enter_context(tc.tile_pool(name="tpool", bufs=2))
    lpool = ctx.enter_context(tc.tile_pool(name="lpool", bufs=2))
    pmm1 = ctx.enter_context(tc.tile_pool(name="pmm1", bufs=2, space="PSUM"))
    pmm2 = ctx.enter_context(tc.tile_pool(name="pmm2", bufs=4, space="PSUM"))

    # ---- inputs / consts ----
    u_sb = consts.tile([H, NW], f32)
    nf_sb = consts.tile([H, NW], f32)
    st_sb = consts.tile([9, NIMG * NG * H], f16)
    sel_sb = consts.tile([9, WG * NB], f16)
    bandch_sb = consts.tile([H + 1, H], f16)
    nc.sync.dma_start(u_sb[:], u_d[:])
    nc.sync.dma_start(nf_sb[:], nf_d[:])
    nc.sync.dma_start(st_sb[:], st_d[:])
    nc.sync.dma_start(sel_sb[:], sel_d[:])
    nc.sync.dma_start(bandch_sb[:], bandch_d[:])

    # two persistent stripes; pads + cw-indicator row initialized via DMA
    stripes = []
    for tag in ("stripeA", "stripeB"):
        s = consts.tile([H + 1, SCOLS], f16, tag=tag)
        nc.sync.dma_start(s[:, 0:2 * NB], padz_d[:])
        nc.sync.dma_start(s[:, SCOLS - 2 * NB:SCOLS], padz_d[:])
        nc.sync.dma_start(s[H:H + 1, 2 * NB:SCOLS - 2 * NB], ones_d[:])
        stripes.append(s)

    # pair-sum stripes: P[j] = t2 block[2j] + block[2j+1] (50 pair blocks);
    # pairs 0 and 49 are static zero pads, indicator row = pairwise count
    NPAIR = NBLK // 2
    pstripes = []
    for tag in ("pairA", "pairB"):
        p = consts.tile([H + 1, NPAIR * NB], f16, tag=tag)
        nc.sync.dma_start(p[:, 0:NB], padz_d[:, 0:NB])
        nc.sync.dma_start(p[:, (NPAIR - 1) * NB:NPAIR * NB], padz_d[:, 0:NB])
        nc.sync.dma_start(p[H:H + 1, NB:(NPAIR - 1) * NB], twos_d[:])
        pstripes.append(p)

    bias_tiles = {}

    def bias_ap(val):
        if val not in bias_tiles:
            t = consts.tile([H, 1], f32, tag=f"bias{val}")
            nc.vector.memset(t[:], val)
            bias_tiles[val] = t
        return bias_tiles[val][:]

    # =====================  S path (tiny, [96, 288])  =====================
    taps = (-2, -1, 0, 1, 2)
    sq = {}
    for o in taps:
        v = sm.tile([H, NW], f32, tag=f"v{o}")
        nc.scalar.activation(v[:], u_sb[:], AF.Tanh, bias=bias_ap(-5.0 * o), scale=5.0)
        nc.scalar.activation(v[:], v[:], AF.Square)
        sq[o] = v
    masks = {}
    for o in taps:
        if o == 0:
            continue
        m = sm.tile([H, NW], f32, tag=f"m{o}")
        if o < 0:
            nc.vector.tensor_scalar(m[:], nf_sb[:], float(-o), None, op0=OP.is_ge)
        else:
            nc.vector.tensor_scalar(m[:], nf_sb[:], float(255 - o), None, op0=OP.is_le)
        masks[o] = m
    cnt = sm.tile([H, NW], f32)
    nc.vector.tensor_tensor(cnt[:], masks[-2][:], masks[-1][:], op=OP.add)
    nc.vector.tensor_tensor(cnt[:], cnt[:], masks[1][:], op=OP.add)
    nc.vector.tensor_tensor(cnt[:], cnt[:], masks[2][:], op=OP.add)
    nc.vector.tensor_scalar(cnt[:], cnt[:], 1.0, None, op0=OP.add)
    ssum = sq[0]
    for o in (-2, -1, 1, 2):
        nc.vector.tensor_tensor(masks[o][:], masks[o][:], sq[o][:], op=OP.mult)
        nc.vector.tensor_tensor(ssum[:], ssum[:], masks[o][:], op=OP.add)
    # spix = cnt - ssum; build negated fp16 copy with a zero 97th row for the
    # -band/ch stationary
    spix = cnt
    nc.vector.tensor_tensor(spix[:], cnt[:], ssum[:], op=OP.subtract)
    sneg = sm.tile([H + 1, NW], f16)
    nc.vector.memset(sneg[H:H + 1, :], 0.0)
    nc.vector.tensor_scalar(sneg[0:H, :], spix[:], -1.0, None, op0=OP.mult)
    ps_s = pmm2.tile([H, 512], f32, tag="ps2")
    nc.tensor.matmul(ps_s[:, 0:NW], bandch_sb[:], sneg[:], start=True, stop=True)
    sh = sm.tile([H, NW], f32)
    nc.scalar.copy(sh[:], ps_s[:, 0:NW])
    shp = sm.tile([H, NIMG, W + 4], f32)
    nc.vector.memset(shp[:], 0.0)
    for i in range(NIMG):
        nc.vector.tensor_copy(shp[:, i, 2:2 + W], sh[:, i * W:(i + 1) * W])
    swin = sm.tile([H, NIMG, W], f32)
    nc.vector.tensor_tensor(swin[:], shp[:, :, 0:W], shp[:, :, 1:1 + W], op=OP.add)
    for j in (2, 3, 4):
        nc.vector.tensor_tensor(swin[:], swin[:], shp[:, :, j:j + W], op=OP.add)
    sw_flat = swin[:].rearrange("p a b -> p (a b)")
    rinv = sm.tile([H, NW], f32)
    nc.vector.tensor_scalar(rinv[:], sw_flat, EPS, None, op0=OP.add)
    nc.vector.reciprocal(rinv[:], rinv[:])
    lnS = sm.tile([H, NW], f32)

    # =====================  main loop  =====================
    # Token tiles create artificial cross-batch deps so the Tile scheduler
    # cannot interleave Tanh and Ln activations (each interleave costs a
    # ~1.3us ACT table load): Ln(i) gates on last tanh(i) via its bias AP,
    # tanh(i+1) gates on last QL column of image i via its bias AP.
    QL = sm.tile([H, NW], f32)
    dummy = sm.tile([H, 1], f32)
    toks = {}

    def emit_group(i, g, tok):
        """MM1 + tanh + square + pair-sum for w-group g of image i."""
        stripe = stripes[i % 2]
        pstripe = pstripes[i % 2]
        gi = i * NG + g
        ps1 = pmm1.tile([H, 1024], f32, tag="ps1")
        stat = st_sb[:, gi * H:(gi + 1) * H]
        nc.tensor.matmul(ps1[:, 0:512], stat, sel_sb[:, 0:512],
                         start=True, stop=True)
        nc.tensor.matmul(ps1[:, 512:1024], stat, sel_sb[:, 512:1024],
                         start=True, stop=True)
        t = tpool.tile([H, 1024], f16, tag="t")
        if tok is None:
            nc.scalar.activation(t[:], ps1[:], AF.Tanh)
        else:
            nc.scalar.activation(t[:], ps1[:], AF.Tanh, bias=tok[:])
        base = (WG * g + 2) * NB
        dst = stripe[0:H, base:base + WG * NB]
        nc.vector.tensor_tensor(dst, t[:], t[:], op=OP.mult)
        # pair sums P[2g+1], P[2g+2]: even/odd interleaved block views
        blk4 = stripe[0:H, base:base + WG * NB].rearrange(
            "p (a c) -> p a c", a=2)
        pdst = pstripe[0:H, (2 * g + 1) * NB:(2 * g + 3) * NB].rearrange(
            "p (a c) -> p a c", c=NB)
        nc.vector.tensor_tensor(
            pdst, blk4[:, :, 0:NB], blk4[:, :, NB:2 * NB], op=OP.add)
        return t

    def make_etok(i, t, tag):
        # eps token: value EPS, data-dependent on a tanh output of image i
        etok = sm.tile([H, 1], f32, tag=tag)
        nc.vector.tensor_scalar(etok[:], t[:, 0:1], 0.0, EPS,
                                op0=OP.mult, op1=OP.add)
        return etok

    def back_chunk(i, c, etok):
        stripe = stripes[i % 2]
        pstripe = pstripes[i % 2]
        ps2 = pmm2.tile([H, 512], f32, tag="ps2")
        # A: pairs (P[c], P[c+1]) serve (even w | odd w); B: (P[c+1],
        # P[c+2]); C: singles blk[2c+4] (even) and blk[2c+1] (odd). B
        # last so the group is opened/closed by full-width matmuls.
        nc.tensor.matmul(ps2[:], bandch_sb[:],
                         pstripe[:, c * NB:(c + 2) * NB],
                         start=True, stop=False)
        nc.tensor.matmul(ps2[:, 0:NB], bandch_sb[:],
                         stripe[:, (2 * c + 4) * NB:(2 * c + 5) * NB],
                         start=False, stop=False, skip_group_check=True)
        nc.tensor.matmul(ps2[:, NB:2 * NB], bandch_sb[:],
                         stripe[:, (2 * c + 1) * NB:(2 * c + 2) * NB],
                         start=False, stop=False, skip_group_check=True)
        nc.tensor.matmul(ps2[:], bandch_sb[:],
                         pstripe[:, (c + 1) * NB:(c + 3) * NB],
                         start=False, stop=True)
        L = lpool.tile([H, 512], f32, tag="L")
        nc.scalar.activation(L[:], ps2[:], AF.Ln, bias=etok[:])
        for j in range(2):
            w = 2 * c + j
            nc.vector.scalar_tensor_tensor(
                dummy.broadcast_to((H, NB)),
                L[:, j * NB:(j + 1) * NB],
                1.0,
                ps2[:, j * NB:(j + 1) * NB],
                op0=OP.mult,
                op1=OP.mult,
                accum_out=QL[:, i * W + w:i * W + w + 1],
            )

    # Per image: two half-batches of tanh and two half-batches of Ln, with
    # token gating so ScalarE runs [T..T][L..L][T..T][L..L] (4 table switches
    # per image) while the PE interleaves second-half MM1 with first-half MM2.
    # Chunk c reads stripe blocks up to 2c+5 and pairs up to c+2, so chunks
    # 0..21 only need groups 0..11; the mid token gates second-half tanhs on
    # the end of the first Ln batch (chunk 21) without a dependency cycle.
    NGH = NG // 2          # 12 groups per half
    NCH = 22               # chunks in first half
    for i in range(NIMG):
        t_last = None
        for g in range(NGH):
            t_last = emit_group(i, g, toks.get(i))
        etok_a = make_etok(i, t_last, f"etokA{i}")
        if i == 0:
            nc.scalar.activation(lnS[:], sw_flat, AF.Ln, bias=etok_a[:])
        for c in range(NCH):
            back_chunk(i, c, etok_a)
        tokmid = sm.tile([H, 1], f32, tag=f"tokM{i}")
        nc.vector.tensor_scalar(
            tokmid[:], QL[:, i * W + 2 * NCH - 1:i * W + 2 * NCH], 0.0, None,
            op0=OP.mult)
        for g in range(NGH, NG):
            t_last = emit_group(i, g, tokmid)
        etok_b = make_etok(i, t_last, f"etokB{i}")
        for c in range(NCH, W // 2):
            back_chunk(i, c, etok_b)
        # tanh token for image i+1: depends on last QL column of image i
        if i + 1 < NIMG:
            tok = sm.tile([H, 1], f32, tag=f"tok{i + 1}")
            nc.vector.tensor_scalar(
                tok[:], QL[:, (i + 1) * W - 1:(i + 1) * W], 0.0, None, op0=OP.mult
            )
            toks[i + 1] = tok

    # E = lnS - QL / S
    ent = sm.tile([H, NW], f32)
    nc.vector.tensor_tensor(ent[:], QL[:], rinv[:], op=OP.mult)
    nc.vector.tensor_tensor(ent[:], lnS[:], ent[:], op=OP.subtract)
    for i in range(NIMG):
        nc.sync.dma_start(ent_d[i], ent[:, i * W:(i + 1) * W])


def _get_compiled():
    if "nc" in _CACHE:
        return _CACHE["nc"]
    from contextlib import ExitStack

    import concourse.tile as tile
    from concourse import bacc, mybir

    f32 = mybir.dt.float32
    f16 = mybir.dt.float16
    nc = bacc.Bacc("TRN2", target_bir_lowering=False, debug=False)
    u_d = nc.dram_tensor("u_sh", [H, NW], f32, kind="ExternalInput").ap()
    nf_d = nc.dram_tensor("nf_sh", [H, NW], f32, kind="ExternalInput").ap()
    st_d = nc.dram_tensor("st_sh", [9, NIMG * NG * H], f16, kind="ExternalInput").ap()
    sel_d = nc.dram_tensor("sel", [9, WG * NB], f16, kind="ExternalInput").ap()
    bandch_d = nc.dram_tensor("bandch", [H + 1, H], f16, kind="ExternalInput").ap()
    padz_d = nc.dram_tensor("padz", [H + 1, 2 * NB], f16, kind="ExternalInput").ap()
    ones_d = nc.dram_tensor("onesmid", [1, W * NB], f16, kind="ExternalInput").ap()
    twos_d = nc.dram_tensor("twosmid", [1, (NBLK // 2 - 2) * NB], f16,
                            kind="ExternalInput").ap()
    ent_d = nc.dram_tensor("ent", [NIMG, H, W], f32, kind="ExternalOutput").ap()

    with tile.TileContext(nc) as tc:
        with ExitStack() as ctx:
            _emit_kernel(
                nc, tc, ctx,
                (u_d, nf_d, st_d, sel_d, bandch_d, padz_d, ones_d, twos_d),
                (ent_d,),
            )
    nc.compile()
    _CACHE["nc"] = nc
    return nc


def make_in_maps(x):
    """x: full [8, 3, 96, 96] -> list of 8 per-core input dicts."""
    x = np.ascontiguousarray(np.asarray(x, dtype=np.float32))
    imgs = x.reshape(NCORES * NIMG, H, W)
    sel, bandch = _build_consts()
    padz = np.zeros((H + 1, 2 * NB), dtype=np.float16)
    onesmid = np.ones((1, W * NB), dtype=np.float16)
    twosmid = np.full((1, (NBLK // 2 - 2) * NB), 2.0, dtype=np.float16)
    in_maps = []
    for c in range(NCORES):
        sh = imgs[c * NIMG:(c + 1) * NIMG]            # [3, 96, 96]
        n = np.trunc(sh)
        u = sh - n
        # [h, i*96+w] layouts for the S path
        u_all = np.ascontiguousarray(u.transpose(1, 0, 2).reshape(H, NW))
        nf_all = np.ascontiguousarray(n.transpose(1, 0, 2).reshape(H, NW))
        # stationary groups: [9, 3*24*96] fp16
        u5t = (5.0 * u).transpose(0, 2, 1).astype(np.float16)   # [3, 96w, 96h]
        n5t = (5.0 * n).transpose(0, 2, 1).astype(np.float16)
        st = np.empty((9, NIMG * NG * H), dtype=np.float16)
        st[8, :] = 1.0
        for i in range(NIMG):
            for g in range(NG):
                col = (i * NG + g) * H
                st[0:WG, col:col + H] = u5t[i, WG * g:WG * g + WG, :]
                st[WG:2 * WG, col:col + H] = n5t[i, WG * g:WG * g + WG, :]
        in_maps.append(
            {
                "u_sh": u_all,
                "nf_sh": nf_all,
                "st_sh": st,
                "sel": sel,
                "bandch": bandch,
                "padz": padz,
                "onesmid": onesmid,
                "twosmid": twosmid,
            }
        )
    return in_maps


def kernel(x):
    """Full inputs in, full outputs out. x: [8, 3, 96, 96] f32."""
    from concourse.bass_utils import run_bass_kernel_spmd

    nc = _get_compiled()
    in_maps = make_in_maps(x)
    res = run_bass_kernel_spmd(nc, in_maps, list(range(NCORES)))
    out = np.stack([res.results[c]["ent"] for c in range(NCORES)])
    return out.reshape(8, 3, H, W).astype(np.float32)


# revision 42
# speedup vs baseline: 1.1831x; 1.1831x over previous
"""Trainium2 Bass kernel for nn_Entropy (histogram_binning): per-pixel Shannon
entropy of a 5x5-window KDE histogram over 256 intensity bins.

Math (validated in numpy to 2.1e-3 max rel err vs f64 oracle):
  k(x,b) = sigmoid'(10(x-b)) = 0.25*(1 - tanh^2(5(x-b)))
  Scale factors cancel in p = q/S, so we use m = 1 - t^2 directly.
  q[h,w,b] = 5x5 window sum of m = cnt(h,w) - winsum5x5(t^2)
  E = -sum_b p ln p = ln(S) - (sum_b q ln(q+EPS))/S,  S = sum_b q
  S comes analytically per pixel from 5 taps of 1-tanh^2 around frac(x)
  (range-masked), then a 5x5 window sum (H via matmul, W via shifted adds).

Pipeline per image (layout: partitions = h, free = (w-block, 256 bins)):
  MM1 (TensorE, fp16): d = 5u + 5n - 5b via stationary [4x 5u^T; 4x 5n^T;
    ones] and a delta-selector moving constant -> PSUM f32. The u/n split
    keeps d exact in fp16 where |d| is small (n,b integers are fp16-exact).
  tanh (ScalarE): PSUM -> SBUF fp16.
  square (DVE, fp16 2x): t^2 -> stripe [97, 100 blocks x 256] with 2+2
    zero-pad w-blocks; row 96 = in-range-w indicator (cw row).
  MM2 (TensorE, fp16, 5 shifted taps accumulated in PSUM): stationary
    [-band(96x96); ch(h)] x stripe -> q = cnt - winsum5x5(t^2) directly in
    PSUM. No scan, no transpose, no separate W pass.
  backend: Ln(q+EPS) (ScalarE, PSUM src), then tensor_tensor_reduce
    (DVE) computes sum_b q*ln per w column -> QL.
  E = lnS - QL/S on [96, 288] tiles.

ScalarE activations are batched per image phase (tanh batch / Ln batch) to
avoid activation-table thrash. Sharding: B*C = 24 images, 3 per core on 8
cores; no collectives. Self-contained; compiled once per process.
"""

import sys

sys.path.insert(0, "/opt/trn_rl_repo")

import numpy as np

H = 96
W = 96
NB = 256
NIMG = 3
NCORES = 8
EPS = 1e-10
WG = 4                 # w's per MM1 group
NG = W // WG           # 24 MM1 groups per image
NBLK = W + 4           # stripe w-blocks incl 2+2 pads
SCOLS = NBLK * NB      # 25600 stripe cols
NW = NIMG * W          # 288
BACKEND = "stt"        # "stt" | "ttr" | "reduce"

_CACHE = {}


def _build_consts():
    # MM1 moving selector [9, WG*NB] fp16: col (w', b) picks stationary rows
    # w' (5u) and 4+w' (5n), plus -5b via the ones row.
    sel = np.zeros((9, WG * NB), dtype=np.float32)
    b = np.arange(NB, dtype=np.float32)
    for j in range(WG):
        sel[j, j * NB:(j + 1) * NB] = 1.0
        sel[4 + j, j * NB:(j + 1) * NB] = 1.0
    sel[8, :] = np.tile(-5.0 * b, WG)

    hh = np.arange(H)
    band = (np.abs(hh[:, None] - hh[None, :]) <= 2).astype(np.float32)
    bandch = np.zeros((H + 1, H), dtype=np.float32)
    bandch[:H, :] = -band
    bandch[H, :] = band.sum(0)  # ch(h) in {3,4,5}
    return sel.astype(np.float16), bandch.astype(np.float16)


def _emit_kernel(nc, tc, ctx, ins, outs):
    from concourse import mybir

    f32 = mybir.dt.float32
    f16 = mybir.dt.float16
    AF = mybir.ActivationFunctionType
    OP = mybir.AluOpType

    u_d, nf_d, st_d, sel_d, bandch_d, padz_d, ones_d, twos_d = ins
    (ent_d,) = outs

    consts = ctx.enter_context(tc.tile_pool(name="consts", bufs=1))
    sm = ctx.enter_context(tc.tile_pool(name="sm", bufs=1))
    tpool = ctx.enter_context(tc.tile_pool(name="tpool", bufs=2))
    lpool = ctx.enter_context(tc.tile_pool(name="lpool", bufs=2))
    pmm1 = ctx.enter_context(tc.tile_pool(name="pmm1", bufs=2, space="PSUM"))
    pmm2 = ctx.enter_context(tc.tile_pool(name="pmm2", bufs=4, space="PSUM"))

    # ---- inputs / consts ----
    u_sb = consts.tile([H, NW], f32)
    nf_sb = consts.tile([H, NW], f32)
    st_sb = consts.tile([9, NIMG * NG * H], f16)
    sel_sb = consts.tile([9, WG * NB], f16)
    bandch_sb = consts.tile([H + 1, H], f16)
    nc.sync.dma_start(u_sb[:], u_d[:])
    nc.sync.dma_start(nf_sb[:], nf_d[:])
    nc.sync.dma_start(st_sb[:], st_d[:])
    nc.sync.dma_start(sel_sb[:], sel_d[:])
    nc.sync.dma_start(bandch_sb[:], bandch_d[:])

    # two persistent stripes; pads + cw-indicator row initialized via DMA
    stripes = []
    for tag in ("stripeA", "stripeB"):
        s = consts.tile([H + 1, SCOLS], f16, tag=tag)
        nc.sync.dma_start(s[:, 0:2 * NB], padz_d[:])
        nc.sync.dma_start(s[:, SCOLS - 2 * NB:SCOLS], padz_d[:])
        nc.sync.dma_start(s[H:H + 1, 2 * NB:SCOLS - 2 * NB], ones_d[:])
        stripes.append(s)

    # pair-sum stripes: P[j] = t2 block[2j] + block[2j+1] (50 pair blocks);
    # pairs 0 and 49 are static zero pads, indicator row = pairwise count
    NPAIR = NBLK // 2
    pstripes = []
    for tag in ("pairA", "pairB"):
        p = consts.tile([H + 1, NPAIR * NB], f16, tag=tag)
        nc.sync.dma_start(p[:, 0:NB], padz_d[:, 0:NB])
        nc.sync.dma_start(p[:, (NPAIR - 1) * NB:NPAIR * NB], padz_d[:, 0:NB])
        nc.sync.dma_start(p[H:H + 1, NB:(NPAIR - 1) * NB], twos_d[:])
        pstripes.append(p)

    bias_tiles = {}

    def bias_ap(val):
        if val not in bias_tiles:
            t = consts.tile([H, 1], f32, tag=f"bias{val}")
            nc.vector.memset(t[:], val)
            bias_tiles[val] = t
        return bias_tiles[val][:]

    # =====================  S path (tiny, [96, 288])  =====================
    taps = (-2, -1, 0, 1, 2)
    sq = {}
    for o in taps:
        v = sm.tile([H, NW], f32, tag=f"v{o}")
        nc.scalar.activation(v[:], u_sb[:], AF.Tanh, bias=bias_ap(-5.0 * o), scale=5.0)
        nc.scalar.activation(v[:], v[:], AF.Square)
        sq[o] = v
    masks = {}
    for o in taps:
        if o == 0:
            continue
        m = sm.tile([H, NW], f32, tag=f"m{o}")
        if o < 0:
            nc.vector.tensor_scalar(m[:], nf_sb[:], float(-o), None, op0=OP.is_ge)
        else:
            nc.vector.tensor_scalar(m[:], nf_sb[:], float(255 - o), None, op0=OP.is_le)
        masks[o] = m
    cnt = sm.tile([H, NW], f32)
    nc.vector.tensor_tensor(cnt[:], masks[-2][:], masks[-1][:], op=OP.add)
    nc.vector.tensor_tensor(cnt[:], cnt[:], masks[1][:], op=OP.add)
    nc.vector.tensor_tensor(cnt[:], cnt[:], masks[2][:], op=OP.add)
    nc.vector.tensor_scalar(cnt[:], cnt[:], 1.0, None, op0=OP.add)
    ssum = sq[0]
    for o in (-2, -1, 1, 2):
        nc.vector.tensor_tensor(masks[o][:], masks[o][:], sq[o][:], op=OP.mult)
        nc.vector.tensor_tensor(ssum[:], ssum[:], masks[o][:], op=OP.add)
    # spix = cnt - ssum; build negated fp16 copy with a zero 97th row for the
    # -band/ch stationary
    spix = cnt
    nc.vector.tensor_tensor(spix[:], cnt[:], ssum[:], op=OP.subtract)
    sneg = sm.tile([H + 1, NW], f16)
    nc.vector.memset(sneg[H:H + 1, :], 0.0)
    nc.vector.tensor_scalar(sneg[0:H, :], spix[:], -1.0, None, op0=OP.mult)
    ps_s = pmm2.tile([H, 512], f32, tag="ps2")
    nc.tensor.matmul(ps_s[:, 0:NW], bandch_sb[:], sneg[:], start=True, stop=True)
    sh = sm.tile([H, NW], f32)
    nc.scalar.copy(sh[:], ps_s[:, 0:NW])
    shp = sm.tile([H, NIMG, W + 4], f32)
    nc.vector.memset(shp[:], 0.0)
    for i in range(NIMG):
        nc.vector.tensor_copy(shp[:, i, 2:2 + W], sh[:, i * W:(i + 1) * W])
    swin = sm.tile([H, NIMG, W], f32)
    nc.vector.tensor_tensor(swin[:], shp[:, :, 0:W], shp[:, :, 1:1 + W], op=OP.add)
    for j in (2, 3, 4):
        nc.vector.tensor_tensor(swin[:], swin[:], shp[:, :, j:j + W], op=OP.add)
    sw_flat = swin[:].rearrange("p a b -> p (a b)")
    rinv = sm.tile([H, NW], f32)
    nc.vector.tensor_scalar(rinv[:], sw_flat, EPS, None, op0=OP.add)
    nc.vector.reciprocal(rinv[:], rinv[:])
    lnS = sm.tile([H, NW], f32)

    # =====================  main loop  =====================
    # Token tiles create artificial cross-batch deps so the Tile scheduler
    # cannot interleave Tanh and Ln activations (each interleave costs a
    # ~1.3us ACT table load): Ln(i) gates on last tanh(i) via its bias AP,
    # tanh(i+1) gates on last QL column of image i via its bias AP.
    QL = sm.tile([H, NW], f32)
    dummy = sm.tile([H, 1], f32)
    toks = {}

    def emit_group(i, g, tok):
        """MM1 + tanh + square + pair-sum for w-group g of image i."""
        stripe = stripes[i % 2]
        pstripe = pstripes[i % 2]
        gi = i * NG + g
        ps1 = pmm1.tile([H, 1024], f32, tag="ps1")
        stat = st_sb[:, gi * H:(gi + 1) * H]
        nc.tensor.matmul(ps1[:, 0:512], stat, sel_sb[:, 0:512],
                         start=True, stop=True)
        nc.tensor.matmul(ps1[:, 512:1024], stat, sel_sb[:, 512:1024],
                         start=True, stop=True)
        t = tpool.tile([H, 1024], f16, tag="t")
        if tok is None:
            nc.scalar.activation(t[:], ps1[:], AF.Tanh)
        else:
            nc.scalar.activation(t[:], ps1[:], AF.Tanh, bias=tok[:])
        base = (WG * g + 2) * NB
        dst = stripe[0:H, base:base + WG * NB]
        nc.vector.tensor_tensor(dst, t[:], t[:], op=OP.mult)
        # pair sums P[2g+1], P[2g+2]: even/odd interleaved block views
        blk4 = stripe[0:H, base:base + WG * NB].rearrange(
            "p (a c) -> p a c", a=2)
        pdst = pstripe[0:H, (2 * g + 1) * NB:(2 * g + 3) * NB].rearrange(
            "p (a c) -> p a c", c=NB)
        nc.vector.tensor_tensor(
            pdst, blk4[:, :, 0:NB], blk4[:, :, NB:2 * NB], op=OP.add)
        return t

    def make_etok(i, t, tag):
        # eps token: value EPS, data-dependent on a tanh output of image i
        etok = sm.tile([H, 1], f32, tag=tag)
        nc.vector.tensor_scalar(etok[:], t[:, 0:1], 0.0, EPS,
                                op0=OP.mult, op1=OP.add)
        return etok

    def back_chunk(i, c, etok):
        stripe = stripes[i % 2]
        pstripe = pstripes[i % 2]
        ps2 = pmm2.tile([H, 512], f32, tag="ps2")
        # A: pairs (P[c], P[c+1]) serve (even w | odd w); B: (P[c+1],
        # P[c+2]); C: singles blk[2c+4] (even) and blk[2c+1] (odd). B
        # last so the group is opened/closed by full-width matmuls.
        nc.tensor.matmul(ps2[:], bandch_sb[:],
                         pstripe[:, c * NB:(c + 2) * NB],
                         start=True, stop=False)
        nc.tensor.matmul(ps2[:, 0:NB], bandch_sb[:],
                         stripe[:, (2 * c + 4) * NB:(2 * c + 5) * NB],
                         start=False, stop=False, skip_group_check=True)
        nc.tensor.matmul(ps2[:, NB:2 * NB], bandch_sb[:],
                         stripe[:, (2 * c + 1) * NB:(2 * c + 2) * NB],
                         start=False, stop=False, skip_group_check=True)
        nc.tensor.matmul(ps2[:], bandch_sb[:],
                         pstripe[:, (c + 1) * NB:(c + 3) * NB],
                         start=False, stop=True)
        L = lpool.tile([H, 512], f32, tag="L")
        nc.scalar.activation(L[:], ps2[:], AF.Ln, bias=etok[:])
        for j in range(2):
            w = 2 * c + j
            nc.vector.scalar_tensor_tensor(
                dummy.broadcast_to((H, NB)),
                ps2[:, j * NB:(j + 1) * NB],
                1.0,
                L[:, j * NB:(j + 1) * NB],
                op0=OP.mult,
                op1=OP.mult,
                accum_out=QL[:, i * W + w:i * W + w + 1],
            )

    # Per image: two half-batches of tanh and two half-batches of Ln, with
    # token gating so ScalarE runs [T..T][L..L][T..T][L..L] (4 table switches
    # per image) while the PE interleaves second-half MM1 with first-half MM2.
    # Chunk c reads stripe blocks up to 2c+5 and pairs up to c+2, so chunks
    # 0..21 only need groups 0..11; the mid token gates second-half tanhs on
    # the end of the first Ln batch (chunk 21) without a dependency cycle.
    NGH = NG // 2          # 12 groups per half
    NCH = 22               # chunks in first half
    for i in range(NIMG):
        t_last = None
        for g in range(NGH):
            t_last = emit_group(i, g, toks.get(i))
        etok_a = make_etok(i, t_last, f"etokA{i}")
        if i == 0:
            nc.scalar.activation(lnS[:], sw_flat, AF.Ln, bias=etok_a[:])
        for c in range(NCH):
            back_chunk(i, c, etok_a)
        tokmid = sm.tile([H, 1], f32, tag=f"tokM{i}")
        nc.vector.tensor_scalar(
            tokmid[:], QL[:, i * W + 2 * NCH - 1:i * W + 2 * NCH], 0.0, None,
            op0=OP.mult)
        for g in range(NGH, NG):
            t_last = emit_group(i, g, tokmid)
        etok_b = make_etok(i, t_last, f"etokB{i}")
        for c in range(NCH, W // 2):
            back_chunk(i, c, etok_b)
        # tanh token for image i+1: depends on last QL column of image i
        if i + 1 < NIMG:
            tok = sm.tile([H, 1], f32, tag=f"tok{i + 1}")
            nc.vector.tensor_scalar(
                tok[:], QL[:, (i + 1) * W - 1:(i + 1) * W], 0.0, None, op0=OP.mult
            )
            toks[i + 1] = tok

    # E = lnS - QL / S
    ent = sm.tile([H, NW], f32)
    nc.vector.tensor_tensor(ent[:], QL[:], rinv[:], op=OP.mult)
    nc.vector.tensor_tensor(ent[:], lnS[:], ent[:], op=OP.subtract)
    for i in range(NIMG):
        nc.sync.dma_start(ent_d[i], ent[:, i * W:(i + 1) * W])


def _get_compiled():
    if "nc" in _CACHE:
        return _CACHE["nc"]
    from contextlib import ExitStack

    import concourse.tile as tile
    from concourse import bacc, mybir

    f32 = mybir.dt.float32
    f16 = mybir.dt.float16
    nc = bacc.Bacc("TRN2", target_bir_lowering=False, debug=False)
    u_d = nc.dram_tensor("u_sh", [H, NW], f32, kind="ExternalInput").ap()
    nf_d = nc.dram_tensor("nf_sh", [H, NW], f32, kind="ExternalInput").ap()
    st_d = nc.dram_tensor("st_sh", [9, NIMG * NG * H], f16, kind="ExternalInput").ap()
    sel_d = nc.dram_tensor("sel", [9, WG * NB], f16, kind="ExternalInput").ap()
    bandch_d = nc.dram_tensor("bandch", [H + 1, H], f16, kind="ExternalInput").ap()
    padz_d = nc.dram_tensor("padz", [H + 1, 2 * NB], f16, kind="ExternalInput").ap()
    ones_d = nc.dram_tensor("onesmid", [1, W * NB], f16, kind="ExternalInput").ap()
    twos_d = nc.dram_tensor("twosmid", [1, (NBLK // 2 - 2) * NB], f16,
                            kind="ExternalInput").ap()
    ent_d = nc.dram_tensor("ent", [NIMG, H, W], f32, kind="ExternalOutput").ap()

    with tile.TileContext(nc) as tc:
        with ExitStack() as ctx:
            _emit_kernel(
                nc, tc, ctx,
                (u_d, nf_d, st_d, sel_d, bandch_d, padz_d, ones_d, twos_d),
                (ent_d,),
            )
    nc.compile()
    _CACHE["nc"] = nc
    return nc


def make_in_maps(x):
    """x: full [8, 3, 96, 96] -> list of 8 per-core input dicts."""
    x = np.ascontiguousarray(np.asarray(x, dtype=np.float32))
    imgs = x.reshape(NCORES * NIMG, H, W)
    sel, bandch = _build_consts()
    padz = np.zeros((H + 1, 2 * NB), dtype=np.float16)
    onesmid = np.ones((1, W * NB), dtype=np.float16)
    twosmid = np.full((1, (NBLK // 2 - 2) * NB), 2.0, dtype=np.float16)
    in_maps = []
    for c in range(NCORES):
        sh = imgs[c * NIMG:(c + 1) * NIMG]            # [3, 96, 96]
        n = np.trunc(sh)
        u = sh - n
        # [h, i*96+w] layouts for the S path
        u_all = np.ascontiguousarray(u.transpose(1, 0, 2).reshape(H, NW))
        nf_all = np.ascontiguousarray(n.transpose(1, 0, 2).reshape(H, NW))
        # stationary groups: [9, 3*24*96] fp16
        u5t = (5.0 * u).transpose(0, 2, 1).astype(np.float16)   # [3, 96w, 96h]
        n5t = (5.0 * n).transpose(0, 2, 1).astype(np.float16)
        st = np.empty((9, NIMG * NG * H), dtype=np.float16)
        st[8, :] = 1.0
        for i in range(NIMG):
            for g in range(NG):
                col = (i * NG + g) * H
                st[0:WG, col:col + H] = u5t[i, WG * g:WG * g + WG, :]
                st[WG:2 * WG, col:col + H] = n5t[i, WG * g:WG * g + WG, :]
        in_maps.append(
            {
                "u_sh": u_all,
                "nf_sh": nf_all,
                "st_sh": st,
                "sel": sel,
                "bandch": bandch,
                "padz": padz,
                "onesmid": onesmid,
                "twosmid": twosmid,
            }
        )
    return in_maps


def kernel(x):
    """Full inputs in, full outputs out. x: [8, 3, 96, 96] f32."""
    from concourse.bass_utils import run_bass_kernel_spmd

    nc = _get_compiled()
    in_maps = make_in_maps(x)
    res = run_bass_kernel_spmd(nc, in_maps, list(range(NCORES)))
    out = np.stack([res.results[c]["ent"] for c in range(NCORES)])
    return out.reshape(8, 3, H, W).astype(np.float32)


# revision 44
# speedup vs baseline: 1.1847x; 1.0014x over previous
"""Trainium2 Bass kernel for nn_Entropy (histogram_binning): per-pixel Shannon
entropy of a 5x5-window KDE histogram over 256 intensity bins.

Math (validated in numpy to 2.1e-3 max rel err vs f64 oracle):
  k(x,b) = sigmoid'(10(x-b)) = 0.25*(1 - tanh^2(5(x-b)))
  Scale factors cancel in p = q/S, so we use m = 1 - t^2 directly.
  q[h,w,b] = 5x5 window sum of m = cnt(h,w) - winsum5x5(t^2)
  E = -sum_b p ln p = ln(S) - (sum_b q ln(q+EPS))/S,  S = sum_b q
  S comes analytically per pixel from 5 taps of 1-tanh^2 around frac(x)
  (range-masked), then a 5x5 window sum (H via matmul, W via shifted adds).

Pipeline per image (layout: partitions = h, free = (w-block, 256 bins)):
  MM1 (TensorE, fp16): d = 5u + 5n - 5b via stationary [4x 5u^T; 4x 5n^T;
    ones] and a delta-selector moving constant -> PSUM f32. The u/n split
    keeps d exact in fp16 where |d| is small (n,b integers are fp16-exact).
  tanh (ScalarE): PSUM -> SBUF fp16.
  square (DVE, fp16 2x): t^2 -> stripe [97, 100 blocks x 256] with 2+2
    zero-pad w-blocks; row 96 = in-range-w indicator (cw row).
  MM2 (TensorE, fp16, 5 shifted taps accumulated in PSUM): stationary
    [-band(96x96); ch(h)] x stripe -> q = cnt - winsum5x5(t^2) directly in
    PSUM. No scan, no transpose, no separate W pass.
  backend: Ln(q+EPS) (ScalarE, PSUM src), then tensor_tensor_reduce
    (DVE) computes sum_b q*ln per w column -> QL.
  E = lnS - QL/S on [96, 288] tiles.

ScalarE activations are batched per image phase (tanh batch / Ln batch) to
avoid activation-table thrash. Sharding: B*C = 24 images, 3 per core on 8
cores; no collectives. Self-contained; compiled once per process.
"""

import sys

sys.path.insert(0, "/opt/trn_rl_repo")

import numpy as np

H = 96
W = 96
NB = 256
NIMG = 3
NCORES = 8
EPS = 1e-10
WG = 4                 # w's per MM1 group
NG = W // WG           # 24 MM1 groups per image
NBLK = W + 4           # stripe w-blocks incl 2+2 pads
SCOLS = NBLK * NB      # 25600 stripe cols
NW = NIMG * W          # 288
BACKEND = "stt"        # "stt" | "ttr" | "reduce"

_CACHE = {}


def _build_consts():
    # MM1 moving selector [9, WG*NB] fp16: col (w', b) picks stationary rows
    # w' (5u) and 4+w' (5n), plus -5b via the ones row.
    sel = np.zeros((9, WG * NB), dtype=np.float32)
    b = np.arange(NB, dtype=np.float32)
    for j in range(WG):
        sel[j, j * NB:(j + 1) * NB] = 1.0
        sel[4 + j, j * NB:(j + 1) * NB] = 1.0
    sel[8, :] = np.tile(-5.0 * b, WG)

    hh = np.arange(H)
    band = (np.abs(hh[:, None] - hh[None, :]) <= 2).astype(np.float32)
    bandch = np.zeros((H + 1, H), dtype=np.float32)
    bandch[:H, :] = -band
    bandch[H, :] = band.sum(0)  # ch(h) in {3,4,5}
    return sel.astype(np.float16), bandch.astype(np.float16)


def _emit_kernel(nc, tc, ctx, ins, outs):
    from concourse import mybir

    f32 = mybir.dt.float32
    f16 = mybir.dt.float16
    AF = mybir.ActivationFunctionType
    OP = mybir.AluOpType

    u_d, nf_d, st_d, sel_d, bandch_d, padz_d, ones_d, twos_d = ins
    (ent_d,) = outs

    consts = ctx.enter_context(tc.tile_pool(name="consts", bufs=1))
    sm = ctx.enter_context(tc.tile_pool(name="sm", bufs=1))
    tpool = ctx.enter_context(tc.tile_pool(name="tpool", bufs=2))
    lpool = ctx.enter_context(tc.tile_pool(name="lpool", bufs=2))
    pmm1 = ctx.enter_context(tc.tile_pool(name="pmm1", bufs=2, space="PSUM"))
    pmm2 = ctx.enter_context(tc.tile_pool(name="pmm2", bufs=4, space="PSUM"))

    # ---- inputs / consts ----
    u_sb = consts.tile([H, NW], f32)
    nf_sb = consts.tile([H, NW], f32)
    st_sb = consts.tile([9, NIMG * NG * H], f16)
    sel_sb = consts.tile([9, WG * NB], f16)
    bandch_sb = consts.tile([H + 1, H], f16)
    nc.sync.dma_start(u_sb[:], u_d[:])
    nc.sync.dma_start(nf_sb[:], nf_d[:])
    nc.sync.dma_start(st_sb[:], st_d[:])
    nc.sync.dma_start(sel_sb[:], sel_d[:])
    nc.sync.dma_start(bandch_sb[:], bandch_d[:])

    # two persistent stripes; pads + cw-indicator row initialized via DMA
    stripes = []
    for tag in ("stripeA", "stripeB"):
        s = consts.tile([H + 1, SCOLS], f16, tag=tag)
        nc.sync.dma_start(s[:, 0:2 * NB], padz_d[:])
        nc.sync.dma_start(s[:, SCOLS - 2 * NB:SCOLS], padz_d[:])
        nc.sync.dma_start(s[H:H + 1, 2 * NB:SCOLS - 2 * NB], ones_d[:])
        stripes.append(s)

    # pair-sum stripes: P[j] = t2 block[2j] + block[2j+1] (50 pair blocks);
    # pairs 0 and 49 are static zero pads, indicator row = pairwise count
    NPAIR = NBLK // 2
    pstripes = []
    for tag in ("pairA", "pairB"):
        p = consts.tile([H + 1, NPAIR * NB], f16, tag=tag)
        nc.sync.dma_start(p[:, 0:NB], padz_d[:, 0:NB])
        nc.sync.dma_start(p[:, (NPAIR - 1) * NB:NPAIR * NB], padz_d[:, 0:NB])
        nc.sync.dma_start(p[H:H + 1, NB:(NPAIR - 1) * NB], twos_d[:])
        pstripes.append(p)

    bias_tiles = {}

    def bias_ap(val):
        if val not in bias_tiles:
            t = consts.tile([H, 1], f32, tag=f"bias{val}")
            nc.vector.memset(t[:], val)
            bias_tiles[val] = t
        return bias_tiles[val][:]

    # =====================  S path (tiny, [96, 288])  =====================
    taps = (-2, -1, 0, 1, 2)
    sq = {}
    for o in taps:
        v = sm.tile([H, NW], f32, tag=f"v{o}")
        nc.scalar.activation(v[:], u_sb[:], AF.Tanh, bias=bias_ap(-5.0 * o), scale=5.0)
        nc.scalar.activation(v[:], v[:], AF.Square)
        sq[o] = v
    masks = {}
    for o in taps:
        if o == 0:
            continue
        m = sm.tile([H, NW], f32, tag=f"m{o}")
        if o < 0:
            nc.vector.tensor_scalar(m[:], nf_sb[:], float(-o), None, op0=OP.is_ge)
        else:
            nc.vector.tensor_scalar(m[:], nf_sb[:], float(255 - o), None, op0=OP.is_le)
        masks[o] = m
    cnt = sm.tile([H, NW], f32)
    nc.vector.tensor_tensor(cnt[:], masks[-2][:], masks[-1][:], op=OP.add)
    nc.vector.tensor_tensor(cnt[:], cnt[:], masks[1][:], op=OP.add)
    nc.vector.tensor_tensor(cnt[:], cnt[:], masks[2][:], op=OP.add)
    nc.vector.tensor_scalar(cnt[:], cnt[:], 1.0, None, op0=OP.add)
    ssum = sq[0]
    for o in (-2, -1, 1, 2):
        nc.vector.tensor_tensor(masks[o][:], masks[o][:], sq[o][:], op=OP.mult)
        nc.vector.tensor_tensor(ssum[:], ssum[:], masks[o][:], op=OP.add)
    # spix = cnt - ssum; build negated fp16 copy with a zero 97th row for the
    # -band/ch stationary
    spix = cnt
    nc.vector.tensor_tensor(spix[:], cnt[:], ssum[:], op=OP.subtract)
    sneg = sm.tile([H + 1, NW], f16)
    nc.vector.memset(sneg[H:H + 1, :], 0.0)
    nc.vector.tensor_scalar(sneg[0:H, :], spix[:], -1.0, None, op0=OP.mult)
    ps_s = pmm2.tile([H, 512], f32, tag="ps2")
    nc.tensor.matmul(ps_s[:, 0:NW], bandch_sb[:], sneg[:], start=True, stop=True)
    sh = sm.tile([H, NW], f32)
    nc.scalar.copy(sh[:], ps_s[:, 0:NW])
    shp = sm.tile([H, NIMG, W + 4], f32)
    nc.vector.memset(shp[:], 0.0)
    for i in range(NIMG):
        nc.vector.tensor_copy(shp[:, i, 2:2 + W], sh[:, i * W:(i + 1) * W])
    swin = sm.tile([H, NIMG, W], f32)
    nc.vector.tensor_tensor(swin[:], shp[:, :, 0:W], shp[:, :, 1:1 + W], op=OP.add)
    for j in (2, 3, 4):
        nc.vector.tensor_tensor(swin[:], swin[:], shp[:, :, j:j + W], op=OP.add)
    sw_flat = swin[:].rearrange("p a b -> p (a b)")
    rinv = sm.tile([H, NW], f32)
    nc.vector.tensor_scalar(rinv[:], sw_flat, EPS, None, op0=OP.add)
    nc.vector.reciprocal(rinv[:], rinv[:])
    lnS = sm.tile([H, NW], f32)

    # =====================  main loop  =====================
    # Token tiles create artificial cross-batch deps so the Tile scheduler
    # cannot interleave Tanh and Ln activations (each interleave costs a
    # ~1.3us ACT table load): Ln(i) gates on last tanh(i) via its bias AP,
    # tanh(i+1) gates on last QL column of image i via its bias AP.
    QL = sm.tile([H, NW], f32)
    dummy = sm.tile([H, 1], f32)
    toks = {}

    def emit_group(i, g, tok):
        """MM1 + tanh + square + pair-sum for w-group g of image i."""
        stripe = stripes[i % 2]
        pstripe = pstripes[i % 2]
        gi = i * NG + g
        ps1 = pmm1.tile([H, 1024], f32, tag="ps1")
        stat = st_sb[:, gi * H:(gi + 1) * H]
        nc.tensor.matmul(ps1[:, 0:512], stat, sel_sb[:, 0:512],
                         start=True, stop=True)
        nc.tensor.matmul(ps1[:, 512:1024], stat, sel_sb[:, 512:1024],
                         start=True, stop=True)
        t = tpool.tile([H, 1024], f16, tag="t")
        if tok is None:
            nc.scalar.activation(t[:], ps1[:], AF.Tanh)
        else:
            nc.scalar.activation(t[:], ps1[:], AF.Tanh, bias=tok[:])
        base = (WG * g + 2) * NB
        dst = stripe[0:H, base:base + WG * NB]
        nc.vector.tensor_tensor(dst, t[:], t[:], op=OP.mult)
        # pair sums P[2g+1], P[2g+2]: even/odd interleaved block views
        blk4 = stripe[0:H, base:base + WG * NB].rearrange(
            "p (a c) -> p a c", a=2)
        pdst = pstripe[0:H, (2 * g + 1) * NB:(2 * g + 3) * NB].rearrange(
            "p (a c) -> p a c", c=NB)
        nc.vector.tensor_tensor(
            pdst, blk4[:, :, 0:NB], blk4[:, :, NB:2 * NB], op=OP.add)
        return t

    def make_etok(i, t, tag):
        # eps token: value EPS, data-dependent on a tanh output of image i
        etok = sm.tile([H, 1], f32, tag=tag)
        nc.vector.tensor_scalar(etok[:], t[:, 0:1], 0.0, EPS,
                                op0=OP.mult, op1=OP.add)
        return etok

    def back_chunk(i, c, etok):
        stripe = stripes[i % 2]
        pstripe = pstripes[i % 2]
        ps2 = pmm2.tile([H, 512], f32, tag="ps2")
        # A: pairs (P[c], P[c+1]) serve (even w | odd w); B: (P[c+1],
        # P[c+2]); C: singles blk[2c+4] (even) and blk[2c+1] (odd). B
        # last so the group is opened/closed by full-width matmuls.
        nc.tensor.matmul(ps2[:], bandch_sb[:],
                         pstripe[:, c * NB:(c + 2) * NB],
                         start=True, stop=False)
        nc.tensor.matmul(ps2[:, 0:NB], bandch_sb[:],
                         stripe[:, (2 * c + 4) * NB:(2 * c + 5) * NB],
                         start=False, stop=False, skip_group_check=True)
        nc.tensor.matmul(ps2[:, NB:2 * NB], bandch_sb[:],
                         stripe[:, (2 * c + 1) * NB:(2 * c + 2) * NB],
                         start=False, stop=False, skip_group_check=True)
        nc.tensor.matmul(ps2[:], bandch_sb[:],
                         pstripe[:, (c + 1) * NB:(c + 3) * NB],
                         start=False, stop=True)
        L = lpool.tile([H, 512], f32, tag="L")
        nc.scalar.activation(L[:], ps2[:], AF.Ln, bias=etok[:])
        for j in range(2):
            w = 2 * c + j
            nc.vector.scalar_tensor_tensor(
                dummy.broadcast_to((H, NB)),
                ps2[:, j * NB:(j + 1) * NB],
                1.0,
                L[:, j * NB:(j + 1) * NB],
                op0=OP.mult,
                op1=OP.mult,
                accum_out=QL[:, i * W + w:i * W + w + 1],
            )

    # Per image: two half-batches of tanh and two half-batches of Ln, with
    # token gating so ScalarE runs [T..T][L..L][T..T][L..L] (4 table switches
    # per image) while the PE interleaves second-half MM1 with first-half MM2.
    # Chunk c reads stripe blocks up to 2c+5 and pairs up to c+2, so chunks
    # 0..21 only need groups 0..11; the mid token gates second-half tanhs on
    # the end of the first Ln batch (chunk 21) without a dependency cycle.
    NGH = NG // 2          # 12 groups per half
    NCH = 22               # chunks in first half
    for i in range(NIMG):
        t_last = None
        for g in range(NGH):
            t_last = emit_group(i, g, toks.get(i))
        etok_a = make_etok(i, t_last, f"etokA{i}")
        if i == 0:
            nc.scalar.activation(lnS[:], sw_flat, AF.Ln, bias=etok_a[:])
        for c in range(NCH):
            back_chunk(i, c, etok_a)
        tokmid = sm.tile([H, 1], f32, tag=f"tokM{i}")
        nc.vector.tensor_scalar(
            tokmid[:], QL[:, i * W + 2 * NCH - 7:i * W + 2 * NCH - 6], 0.0, None,
            op0=OP.mult)
        for g in range(NGH, NG):
            t_last = emit_group(i, g, tokmid)
        etok_b = make_etok(i, t_last, f"etokB{i}")
        for c in range(NCH, W // 2):
            back_chunk(i, c, etok_b)
        # tanh token for image i+1: depends on last QL column of image i
        if i + 1 < NIMG:
            tok = sm.tile([H, 1], f32, tag=f"tok{i + 1}")
            nc.vector.tensor_scalar(
                tok[:], QL[:, (i + 1) * W - 7:(i + 1) * W - 6], 0.0, None,
                op0=OP.mult
            )
            toks[i + 1] = tok

    # E = lnS - QL / S
    ent = sm.tile([H, NW], f32)
    nc.vector.tensor_tensor(ent[:], QL[:], rinv[:], op=OP.mult)
    nc.vector.tensor_tensor(ent[:], lnS[:], ent[:], op=OP.subtract)
    for i in range(NIMG):
        nc.sync.dma_start(ent_d[i], ent[:, i * W:(i + 1) * W])


def _get_compiled():
    if "nc" in _CACHE:
        return _CACHE["nc"]
    from contextlib import ExitStack

    import concourse.tile as tile
    from concourse import bacc, mybir

    f32 = mybir.dt.float32
    f16 = mybir.dt.float16
    nc = bacc.Bacc("TRN2", target_bir_lowering=False, debug=False)
    u_d = nc.dram_tensor("u_sh", [H, NW], f32, kind="ExternalInput").ap()
    nf_d = nc.dram_tensor("nf_sh", [H, NW], f32, kind="ExternalInput").ap()
    st_d = nc.dram_tensor("st_sh", [9, NIMG * NG * H], f16, kind="ExternalInput").ap()
    sel_d = nc.dram_tensor("sel", [9, WG * NB], f16, kind="ExternalInput").ap()
    bandch_d = nc.dram_tensor("bandch", [H + 1, H], f16, kind="ExternalInput").ap()
    padz_d = nc.dram_tensor("padz", [H + 1, 2 * NB], f16, kind="ExternalInput").ap()
    ones_d = nc.dram_tensor("onesmid", [1, W * NB], f16, kind="ExternalInput").ap()
    twos_d = nc.dram_tensor("twosmid", [1, (NBLK // 2 - 2) * NB], f16,
                            kind="ExternalInput").ap()
    ent_d = nc.dram_tensor("ent", [NIMG, H, W], f32, kind="ExternalOutput").ap()

    with tile.TileContext(nc) as tc:
        with ExitStack() as ctx:
            _emit_kernel(
                nc, tc, ctx,
                (u_d, nf_d, st_d, sel_d, bandch_d, padz_d, ones_d, twos_d),
                (ent_d,),
            )
    nc.compile()
    _CACHE["nc"] = nc
    return nc


def make_in_maps(x):
    """x: full [8, 3, 96, 96] -> list of 8 per-core input dicts."""
    x = np.ascontiguousarray(np.asarray(x, dtype=np.float32))
    imgs = x.reshape(NCORES * NIMG, H, W)
    sel, bandch = _build_consts()
    padz = np.zeros((H + 1, 2 * NB), dtype=np.float16)
    onesmid = np.ones((1, W * NB), dtype=np.float16)
    twosmid = np.full((1, (NBLK // 2 - 2) * NB), 2.0, dtype=np.float16)
    in_maps = []
    for c in range(NCORES):
        sh = imgs[c * NIMG:(c + 1) * NIMG]            # [3, 96, 96]
        n = np.trunc(sh)
        u = sh - n
        # [h, i*96+w] layouts for the S path
        u_all = np.ascontiguousarray(u.transpose(1, 0, 2).reshape(H, NW))
        nf_all = np.ascontiguousarray(n.transpose(1, 0, 2).reshape(H, NW))
        # stationary groups: [9, 3*24*96] fp16
        u5t = (5.0 * u).transpose(0, 2, 1).astype(np.float16)   # [3, 96w, 96h]
        n5t = (5.0 * n).transpose(0, 2, 1).astype(np.float16)
        st = np.empty((9, NIMG * NG * H), dtype=np.float16)
        st[8, :] = 1.0
        for i in range(NIMG):
            for g in range(NG):
                col = (i * NG + g) * H
                st[0:WG, col:col + H] = u5t[i, WG * g:WG * g + WG, :]
                st[WG:2 * WG, col:col + H] = n5t[i, WG * g:WG * g + WG, :]
        in_maps.append(
            {
                "u_sh": u_all,
                "nf_sh": nf_all,
                "st_sh": st,
                "sel": sel,
                "bandch": bandch,
                "padz": padz,
                "onesmid": onesmid,
                "twosmid": twosmid,
            }
        )
    return in_maps


def kernel(x):
    """Full inputs in, full outputs out. x: [8, 3, 96, 96] f32."""
    from concourse.bass_utils import run_bass_kernel_spmd

    nc = _get_compiled()
    in_maps = make_in_maps(x)
    res = run_bass_kernel_spmd(nc, in_maps, list(range(NCORES)))
    out = np.stack([res.results[c]["ent"] for c in range(NCORES)])
    return out.reshape(8, 3, H, W).astype(np.float32)
